# revision 5
# baseline (speedup 1.0000x reference)
"""Causal multi-head attention (B=2, S=2048, D=1024, H=16) on 8 Trainium2
NeuronCores, tensor-parallel over heads (2 heads per core).

Strategy per core c (heads 2c, 2c+1):
  - host pre-transposes x -> xT [D, B*S] and slices Wq/Wk/Wv columns,
    Wo rows for the core.
  - QT = Wq_c^T x^T, KT = Wk_c^T x^T, VT = Wv_c^T x^T   ([128, 4096], fp32r)
  - V' tiles [128 tok, 65] = PE-transpose of VT plus a ones column (the ones
    column makes the PV matmul accumulate the softmax denominator as row 64).
  - ST = K QT per (128-key-tile x 512-query-tile), both heads row-packed on
    the PE array; exp(scale*ST) on ScalarE straight out of PSUM; causal ramp
    masks multiplied in on the diagonal tiles; OT[65,512] += V'^T PT in PSUM.
  - normalize columns by 1/l (l = OT row 64) via reciprocal + K=1 broadcast
    matmul; output projection out_partial = OT^T Wo_c -> [4096, 1024].
  - host sums the 8 partial outputs (row-sharded Wo => partials add up).

All matmuls run in float32r (full PE rate for free dim >= 256, ~2^-13
relative rounding).
"""

import sys

sys.path.insert(0, "/opt/trn_rl_repo")

import numpy as np

import concourse.bass as bass
import concourse.tile as tile
from concourse import mybir
from concourse import bass_utils

F32 = mybir.dt.float32
F32R = mybir.dt.float32r
EXP = mybir.ActivationFunctionType.Exp

B, S, D, H = 2, 2048, 1024, 16
T = B * S                      # 4096 tokens
DH = 64                        # head dim
NCORES = 8
HPC = H // NCORES              # 2 heads per core
DC = HPC * DH                  # 128 dims per core
SCALE = float(D) ** -0.5       # 1/32 (matches the reference's full-dim scale)

NT = T // 512                  # 8 token tiles of 512
JT = T // 128                  # 32 key tiles of 128
ITPB = S // 512                # 4 query tiles per batch
JTPB = S // 128                # 16 key tiles per batch


def _split_waits(nc):
    """This walrus build rejects >1 sync-wait per instruction; hoist extras
    onto same-engine NoOps placed immediately before (engines execute their
    instructions in block order, so semantics are unchanged)."""
    ctr = 0
    for f in nc.m.functions:
        for b in f.blocks:
            out = []
            changed = False
            for inst in b.instructions:
                si = inst.sync_info
                if si is not None:
                    waits = list(si.on_wait)
                    if len(waits) > 1:
                        for w in waits[:-1]:
                            ctr += 1
                            out.append(
                                mybir.InstNoOp(
                                    name=f"waitsplit-{ctr}",
                                    opcode="NoOp",
                                    engine=inst.engine,
                                    ins=[],
                                    outs=[],
                                    sync_info=mybir.SyncInfo(
                                        on_wait=[w], on_update=[]
                                    ),
                                )
                            )
                        inst.sync_info = mybir.SyncInfo(
                            on_wait=waits[-1:], on_update=list(si.on_update)
                        )
                        changed = True
                out.append(inst)
            if changed:
                b.instructions = out


def _build():
    nc = bass.Bass("TRN2", target_bir_lowering=False, debug=False)

    xt_d = nc.dram_tensor("xt", [D, T], F32R, kind="ExternalInput").ap()
    wq_d = nc.dram_tensor("wq", [D, DC], F32R, kind="ExternalInput").ap()
    wk_d = nc.dram_tensor("wk", [D, DC], F32R, kind="ExternalInput").ap()
    wv_d = nc.dram_tensor("wv", [D, DC], F32R, kind="ExternalInput").ap()
    wo_d = nc.dram_tensor("wo", [DC, D], F32R, kind="ExternalInput").ap()
    mask_d = nc.dram_tensor("mask", [2, 128, 2048], F32R, kind="ExternalInput").ap()
    ident_d = nc.dram_tensor("ident", [128, DH], F32, kind="ExternalInput").ap()
    onescol_d = nc.dram_tensor("onescol", [128, 1], F32R, kind="ExternalInput").ap()
    ones1_d = nc.dram_tensor("ones1", [1, DH], F32R, kind="ExternalInput").ap()
    out_d = nc.dram_tensor("out", [T, D], F32, kind="ExternalOutput").ap()

    with tile.TileContext(nc) as tc:
        with (
            tc.tile_pool(name="const", bufs=1) as cpool,
            tc.tile_pool(name="big", bufs=1) as big,
            tc.tile_pool(name="xtp", bufs=2) as xtp,
            tc.tile_pool(name="vstage", bufs=2) as vstage,
            tc.tile_pool(name="ptp", bufs=2) as ptp,
            tc.tile_pool(name="otnp", bufs=2) as otnp,
            tc.tile_pool(name="lrow", bufs=4) as lrow,
            tc.tile_pool(name="outsb", bufs=3) as outsb,
            tc.tile_pool(name="pp", bufs=2, space="PSUM") as pp,
            tc.tile_pool(name="stp", bufs=1, space="PSUM") as stp,
            tc.tile_pool(name="otp", bufs=2, space="PSUM") as otp,
        ):
            # --- constants / weights resident in SBUF
            wq = cpool.tile([128, 8, DC], F32R, tag="wq")
            wk = cpool.tile([128, 8, DC], F32R, tag="wk")
            wv = cpool.tile([128, 8, DC], F32R, tag="wv")
            wo = cpool.tile([DC, D], F32R, tag="wo")
            masks = cpool.tile([128, 2, 2048], F32R, tag="masks")
            ident = cpool.tile([128, DH], F32, tag="ident")
            onescol = cpool.tile([128, 1], F32R, tag="onescol")
            ones1 = cpool.tile([1, DH], F32R, tag="ones1")
            nc.sync.dma_start(wq[:], wq_d.rearrange("(a p) n -> p a n", p=128))
            nc.sync.dma_start(wk[:], wk_d.rearrange("(a p) n -> p a n", p=128))
            nc.sync.dma_start(wv[:], wv_d.rearrange("(a p) n -> p a n", p=128))
            nc.sync.dma_start(wo[:], wo_d[:])
            nc.sync.dma_start(masks[:], mask_d.rearrange("v p n -> p v n"))
            nc.sync.dma_start(ident[:], ident_d[:])
            nc.sync.dma_start(onescol[:], onescol_d[:])
            nc.sync.dma_start(ones1[:], ones1_d[:])

            qt = big.tile([128, T], F32R, tag="qt")
            kt = big.tile([128, T], F32R, tag="kt")
            vp = big.tile([128, 2 * JT, 65], F32R, tag="vp")

            # --- projections: per 512-token tile, QT/KT/VT then V' transposes
            for n in range(NT):
                xt = xtp.tile([128, 8, 512], F32R, tag="xt")
                nc.sync.dma_start(
                    xt[:],
                    xt_d[:, n * 512 : (n + 1) * 512].rearrange(
                        "(a p) n -> p a n", p=128
                    ),
                )
                tok = slice(n * 512, (n + 1) * 512)
                for w_sb, dst in ((wq, qt), (wk, kt)):
                    ps = pp.tile([128, 512], F32, tag="pp")
                    for k in range(8):
                        nc.tensor.matmul(
                            ps[:],
                            w_sb[:, k, :],
                            xt[:, k, :],
                            start=(k == 0),
                            stop=(k == 7),
                        )
                    nc.scalar.copy(dst[:, tok], ps[:])
                ps = pp.tile([128, 512], F32, tag="pp")
                for k in range(8):
                    nc.tensor.matmul(
                        ps[:], wv[:, k, :], xt[:, k, :],
                        start=(k == 0), stop=(k == 7),
                    )
                vt = vstage.tile([128, 512], F32, tag="vt")
                nc.scalar.copy(vt[:], ps[:])
                # V' tiles: transpose each [64,128] block -> [128,64] (+ones)
                for jj in range(4):
                    jt = n * 4 + jj
                    for hh in range(2):
                        tp = pp.tile([128, 512], F32, tag="pp")
                        nc.tensor.transpose(
                            tp[:, 0:DH],
                            vt[hh * DH : (hh + 1) * DH, jj * 128 : (jj + 1) * 128],
                            ident[hh * DH : (hh + 1) * DH, :],
                        )
                        idx = jt * 2 + hh
                        nc.vector.tensor_copy(vp[:, idx, 0:DH], tp[:, 0:DH])
                        nc.vector.tensor_copy(vp[:, idx, DH:65], onescol[:])

            # --- attention + output projection, per batch / query tile
            for b in range(B):
                for t in range(ITPB):
                    g = b * ITPB + t          # global i-tile
                    i0 = g * 512
                    otn = otnp.tile([128, 512], F32R, tag="otn")
                    ot_h = [
                        otp.tile([65, 512], F32, tag="oth", name=f"ot_{g}_{hh}")
                        for hh in range(2)
                    ]
                    ngrp = 2 * (t + 1)        # groups of 2 key tiles
                    for gg in range(ngrp):
                        st = stp.tile([128, 4, 512], F32, tag="st")
                        # ST: both heads row-packed, interleaved issue order
                        for kk in range(2):
                            jl = 2 * gg + kk            # j-tile within batch
                            jt = b * JTPB + jl
                            for hh in range(2):
                                hs = slice(hh * DH, (hh + 1) * DH)
                                nc.tensor.matmul(
                                    st[:, 2 * hh + kk, :],
                                    kt[hs, jt * 128 : (jt + 1) * 128],
                                    qt[hs, i0 : i0 + 512],
                                    start=True,
                                    stop=True,
                                    tile_position=(hh * DH, 0),
                                )
                        pt = ptp.tile([128, 4, 512], F32R, tag="pt")
                        nc.scalar.activation(
                            pt[:].rearrange("p a n -> p (a n)"),
                            st[:].rearrange("p a n -> p (a n)"),
                            EXP,
                            scale=SCALE,
                        )
                        if gg >= 2 * t:  # diagonal group -> causal ramp mask
                            nc.vector.tensor_tensor(
                                pt[:].rearrange("p a n -> p (a n)"),
                                pt[:].rearrange("p a n -> p (a n)"),
                                masks[:, gg - 2 * t, :],
                                mybir.AluOpType.mult,
                            )
                        first = gg == 0
                        last = gg == ngrp - 1
                        for kk in range(2):
                            jl = 2 * gg + kk
                            jt = b * JTPB + jl
                            for hh in range(2):
                                nc.tensor.matmul(
                                    ot_h[hh][:],
                                    vp[:, jt * 2 + hh, :],
                                    pt[:, 2 * hh + kk, :],
                                    start=(first and kk == 0),
                                    stop=(last and kk == 1),
                                )
                    # normalize: otn[hh*64:(hh+1)*64] = OT * (1/l) broadcast
                    for hh in range(2):
                        linv = lrow.tile([1, 512], F32, tag="linv")
                        nc.vector.reciprocal(linv[:], ot_h[hh][64:65, :])
                        linvr = lrow.tile([1, 512], F32R, tag="linvr")
                        nc.scalar.copy(linvr[:], linv[:])
                        lb = pp.tile([128, 512], F32, tag="pp")
                        nc.tensor.matmul(
                            lb[0:DH, :], ones1[:], linvr[:], start=True, stop=True
                        )
                        otsb = lrow.tile([DH, 512], F32, tag="otsb")
                        nc.scalar.copy(otsb[:], ot_h[hh][0:DH, :])
                        nc.vector.tensor_tensor(
                            otn[hh * DH : (hh + 1) * DH, :],
                            otsb[:],
                            lb[0:DH, :],
                            mybir.AluOpType.mult,
                        )
                    # output projection for this i-tile
                    for c in range(4):
                        for ncol in range(2):
                            op = pp.tile([128, 512], F32, tag="pp")
                            nc.tensor.matmul(
                                op[:],
                                otn[:, c * 128 : (c + 1) * 128],
                                wo[:, ncol * 512 : (ncol + 1) * 512],
                                start=True,
                                stop=True,
                            )
                            osb = outsb.tile([128, 512], F32, tag="osb")
                            nc.vector.tensor_copy(osb[:], op[:])
                            nc.sync.dma_start(
                                out_d[
                                    i0 + c * 128 : i0 + (c + 1) * 128,
                                    ncol * 512 : (ncol + 1) * 512,
                                ],
                                osb[:],
                            )

    _split_waits(nc)
    return nc


_NC = None


def _get_nc():
    global _NC
    if _NC is None:
        _NC = _build()
    return _NC


def _ramp_masks():
    """masks[v][p, k*512 + f] for group-slot layout [h0j0, h0j1, h1j0, h1j1]:
    slot kk (j-tile offset delta = 512*v + 128*kk): allow f >= delta + p."""
    f = np.arange(512)[None, :]
    p = np.arange(128)[:, None]
    m = np.zeros((2, 128, 4, 512), np.float32)
    for v in range(2):
        for kk in range(2):
            delta = 256 * v + 128 * kk
            allow = (f >= delta + p).astype(np.float32)
            for hh in range(2):
                m[v, :, 2 * hh + kk, :] = allow
    return m.reshape(2, 128, 2048)


def _reference_numpy(x, Wq, bq, Wk, bk, Wv, bv, Wo, bo):
    B_, S_, D_ = x.shape
    d = D_ // H
    x64 = x.astype(np.float64)
    q = (x64 @ Wq + bq).reshape(B_, S_, H, d).transpose(0, 2, 1, 3)
    k = (x64 @ Wk + bk).reshape(B_, S_, H, d).transpose(0, 2, 1, 3)
    v = (x64 @ Wv + bv).reshape(B_, S_, H, d).transpose(0, 2, 1, 3)
    dots = np.einsum("bhid,bhjd->bhij", q, k) * (D_ ** -0.5)
    mask = np.triu(np.ones((S_, S_), bool), k=1)
    dots = np.where(mask, -np.inf, dots)
    dots -= dots.max(axis=-1, keepdims=True)
    e = np.exp(dots)
    attn = e / e.sum(axis=-1, keepdims=True)
    out = np.einsum("bhij,bhjd->bhid", attn, v)
    out = out.transpose(0, 2, 1, 3).reshape(B_, S_, D_)
    return (out @ Wo + bo).astype(np.float32)


def kernel(x, Wq, bq, Wk, bk, Wv, bv, Wo, bo):
    x = np.asarray(x, np.float32)
    Wq, Wk, Wv, Wo = (np.asarray(w, np.float32) for w in (Wq, Wk, Wv, Wo))
    bq, bk, bv, bo = (np.asarray(b_, np.float32) for b_ in (bq, bk, bv, bo))
    if np.any(bq) or np.any(bk) or np.any(bv):
        # projection biases feed the softmax nonlinearly; the fused kernel
        # hardcodes zero biases (as in the problem inputs), so fall back
        return _reference_numpy(x, Wq, bq, Wk, bk, Wv, bv, Wo, bo)

    nc = _get_nc()
    xt = np.ascontiguousarray(x.reshape(T, D).T)
    masks = _ramp_masks()
    ident = np.concatenate([np.eye(DH, dtype=np.float32)] * 2, axis=0)
    onescol = np.ones((128, 1), np.float32)
    ones1 = np.ones((1, DH), np.float32)

    in_maps = []
    for c in range(NCORES):
        cs = slice(c * DC, (c + 1) * DC)
        in_maps.append(
            {
                "xt": xt,
                "wq": np.ascontiguousarray(Wq[:, cs]),
                "wk": np.ascontiguousarray(Wk[:, cs]),
                "wv": np.ascontiguousarray(Wv[:, cs]),
                "wo": np.ascontiguousarray(Wo[cs, :]),
                "mask": masks,
                "ident": ident,
                "onescol": onescol,
                "ones1": ones1,
            }
        )
    res = bass_utils.run_bass_kernel_spmd(nc, in_maps, core_ids=list(range(NCORES)))
    out = np.zeros((T, D), np.float64)
    for c in range(NCORES):
        out += res.results[c]["out"]
    out += bo
    return out.astype(np.float32).reshape(B, S, D)


# revision 14
# speedup vs baseline: 1.6472x; 1.6472x over previous
"""Causal multi-head attention (B=2, S=2048, D=1024, H=16) on 8 Trainium2
NeuronCores, tensor-parallel over heads (2 heads per core).

Strategy per core c (heads 2c, 2c+1):
  - host pre-transposes x -> xT [D, B*S] and slices Wq/Wk/Wv columns,
    Wo rows for the core.
  - QT = Wq_c^T x^T, KT = Wk_c^T x^T, VT = Wv_c^T x^T   ([128, 4096], fp32r)
  - V' tiles [128 tok, 65] = PE-transpose of VT plus a ones column (the ones
    column makes the PV matmul accumulate the softmax denominator as row 64).
  - ST = K QT per (128-key-tile x 512-query-tile), both heads row-packed on
    the PE array; exp(scale*ST) on ScalarE straight out of PSUM; causal ramp
    masks multiplied in on the diagonal tiles; OT[65,512] += V'^T PT in PSUM.
  - normalize columns by 1/l (l = OT row 64) via reciprocal + K=1 broadcast
    matmul; output projection out_partial = OT^T Wo_c -> [4096, 1024].
  - host sums the 8 partial outputs (row-sharded Wo => partials add up).

All matmuls run in float32r (full PE rate for free dim >= 256, ~2^-13
relative rounding).
"""

import sys

sys.path.insert(0, "/opt/trn_rl_repo")

import numpy as np

import concourse.bass as bass
import concourse.tile as tile
from concourse import mybir
from concourse import bass_utils

F32 = mybir.dt.float32
F32R = mybir.dt.float32r
EXP = mybir.ActivationFunctionType.Exp

B, S, D, H = 2, 2048, 1024, 16
T = B * S                      # 4096 tokens
DH = 64                        # head dim
NCORES = 8
HPC = H // NCORES              # 2 heads per core
DC = HPC * DH                  # 128 dims per core
SCALE = float(D) ** -0.5       # 1/32 (matches the reference's full-dim scale)

NT = T // 512                  # 8 token tiles of 512
JT = T // 128                  # 32 key tiles of 128
ITPB = S // 512                # 4 query tiles per batch
JTPB = S // 128                # 16 key tiles per batch


def _split_waits(nc):
    """This walrus build rejects >1 sync-wait per instruction; hoist extras
    onto same-engine NoOps placed immediately before (engines execute their
    instructions in block order, so semantics are unchanged)."""
    ctr = 0
    for f in nc.m.functions:
        for b in f.blocks:
            out = []
            changed = False
            for inst in b.instructions:
                si = inst.sync_info
                if si is not None:
                    waits = list(si.on_wait)
                    if len(waits) > 1:
                        for w in waits[:-1]:
                            ctr += 1
                            out.append(
                                mybir.InstNoOp(
                                    name=f"waitsplit-{ctr}",
                                    opcode="NoOp",
                                    engine=inst.engine,
                                    ins=[],
                                    outs=[],
                                    sync_info=mybir.SyncInfo(
                                        on_wait=[w], on_update=[]
                                    ),
                                )
                            )
                        inst.sync_info = mybir.SyncInfo(
                            on_wait=waits[-1:], on_update=list(si.on_update)
                        )
                        changed = True
                out.append(inst)
            if changed:
                b.instructions = out


def _build():
    nc = bass.Bass("TRN2", target_bir_lowering=False, debug=False)

    xt_d = nc.dram_tensor("xt", [D, T], F32R, kind="ExternalInput").ap()
    wq_d = nc.dram_tensor("wq", [D, DC], F32R, kind="ExternalInput").ap()
    wk_d = nc.dram_tensor("wk", [D, DC], F32R, kind="ExternalInput").ap()
    wv_d = nc.dram_tensor("wv", [D, DC], F32R, kind="ExternalInput").ap()
    wo_d = nc.dram_tensor("wo", [DC, D], F32R, kind="ExternalInput").ap()
    mask_d = nc.dram_tensor("mask", [2, 128, 2048], F32R, kind="ExternalInput").ap()
    ident_d = nc.dram_tensor("ident", [128, DH], F32, kind="ExternalInput").ap()
    onescol_d = nc.dram_tensor("onescol", [128, 1], F32R, kind="ExternalInput").ap()
    ones1_d = nc.dram_tensor("ones1", [1, DH], F32R, kind="ExternalInput").ap()
    out_d = nc.dram_tensor("out", [T, D], F32, kind="ExternalOutput").ap()

    with tile.TileContext(nc) as tc:
        with (
            tc.tile_pool(name="const", bufs=1) as cpool,
            tc.tile_pool(name="big", bufs=1) as big,
            tc.tile_pool(name="xtp", bufs=2) as xtp,
            tc.tile_pool(name="vstage", bufs=2) as vstage,
            tc.tile_pool(name="ptp", bufs=2) as ptp,
            tc.tile_pool(name="otnp", bufs=2) as otnp,
            tc.tile_pool(name="lrow", bufs=4) as lrow,
            tc.tile_pool(name="outsb", bufs=3) as outsb,
            tc.tile_pool(name="pp", bufs=2, space="PSUM") as pp,
            tc.tile_pool(name="stp", bufs=1, space="PSUM") as stp,
            tc.tile_pool(name="otp", bufs=2, space="PSUM") as otp,
        ):
            # --- constants / weights resident in SBUF
            wq = cpool.tile([128, 8, DC], F32R, tag="wq")
            wk = cpool.tile([128, 8, DC], F32R, tag="wk")
            wv = cpool.tile([128, 8, DC], F32R, tag="wv")
            wo = cpool.tile([DC, D], F32R, tag="wo")
            masks = cpool.tile([128, 2, 2048], F32R, tag="masks")
            ident = cpool.tile([128, DH], F32, tag="ident")
            onescol = cpool.tile([128, 1], F32R, tag="onescol")
            ones1 = cpool.tile([1, DH], F32R, tag="ones1")
            nc.sync.dma_start(wq[:], wq_d.rearrange("(a p) n -> p a n", p=128))
            nc.sync.dma_start(wk[:], wk_d.rearrange("(a p) n -> p a n", p=128))
            nc.sync.dma_start(wv[:], wv_d.rearrange("(a p) n -> p a n", p=128))
            nc.sync.dma_start(wo[:], wo_d[:])
            nc.sync.dma_start(masks[:], mask_d.rearrange("v p n -> p v n"))
            nc.sync.dma_start(ident[:], ident_d[:])
            nc.sync.dma_start(onescol[:], onescol_d[:])
            nc.sync.dma_start(ones1[:], ones1_d[:])

            qt = big.tile([128, T], F32R, tag="qt")
            kt = big.tile([128, T], F32R, tag="kt")
            vp = big.tile([128, 2 * JT, 65], F32R, tag="vp")

            # --- projections: per 512-token tile, QT/KT/VT then V' transposes
            for n in range(NT):
                xt = xtp.tile([128, 8, 512], F32R, tag="xt")
                nc.sync.dma_start(
                    xt[:],
                    xt_d[:, n * 512 : (n + 1) * 512].rearrange(
                        "(a p) n -> p a n", p=128
                    ),
                )
                tok = slice(n * 512, (n + 1) * 512)
                for w_sb, dst in ((wq, qt), (wk, kt)):
                    ps = pp.tile([128, 512], F32, tag="pp")
                    for k in range(8):
                        nc.tensor.matmul(
                            ps[:],
                            w_sb[:, k, :],
                            xt[:, k, :],
                            start=(k == 0),
                            stop=(k == 7),
                        )
                    nc.scalar.copy(dst[:, tok], ps[:])
                ps = pp.tile([128, 512], F32, tag="pp")
                for k in range(8):
                    nc.tensor.matmul(
                        ps[:], wv[:, k, :], xt[:, k, :],
                        start=(k == 0), stop=(k == 7),
                    )
                vt = vstage.tile([128, 512], F32, tag="vt")
                nc.scalar.copy(vt[:], ps[:])
                # V' tiles: transpose each [64,128] block -> [128,64] (+ones)
                for jj in range(4):
                    jt = n * 4 + jj
                    for hh in range(2):
                        tp = pp.tile([128, 512], F32, tag="pp")
                        nc.tensor.transpose(
                            tp[:, 0:DH],
                            vt[hh * DH : (hh + 1) * DH, jj * 128 : (jj + 1) * 128],
                            ident[hh * DH : (hh + 1) * DH, :],
                        )
                        idx = jt * 2 + hh
                        nc.vector.tensor_copy(vp[:, idx, 0:DH], tp[:, 0:DH])
                        nc.vector.tensor_copy(vp[:, idx, DH:65], onescol[:])

            # --- attention + output projection, per batch / query tile
            for b in range(B):
                for t in range(ITPB):
                    g = b * ITPB + t          # global i-tile
                    i0 = g * 512
                    otn = otnp.tile([128, 512], F32R, tag="otn")
                    ot_h = [
                        otp.tile([65, 512], F32, tag="oth", name=f"ot_{g}_{hh}")
                        for hh in range(2)
                    ]
                    ngrp = 2 * (t + 1)        # groups of 2 key tiles
                    for gg in range(ngrp):
                        st = stp.tile([128, 4, 512], F32, tag="st")
                        # ST: both heads row-packed, interleaved issue order
                        for kk in range(2):
                            jl = 2 * gg + kk            # j-tile within batch
                            jt = b * JTPB + jl
                            for hh in range(2):
                                hs = slice(hh * DH, (hh + 1) * DH)
                                nc.tensor.matmul(
                                    st[:, 2 * hh + kk, :],
                                    kt[hs, jt * 128 : (jt + 1) * 128],
                                    qt[hs, i0 : i0 + 512],
                                    start=True,
                                    stop=True,
                                    tile_position=(hh * DH, 0),
                                )
                        pt = ptp.tile([128, 4, 512], F32R, tag="pt")
                        nc.scalar.activation(
                            pt[:].rearrange("p a n -> p (a n)"),
                            st[:].rearrange("p a n -> p (a n)"),
                            EXP,
                            scale=SCALE,
                        )
                        if gg >= 2 * t:  # diagonal group -> causal ramp mask
                            nc.vector.tensor_tensor(
                                pt[:].rearrange("p a n -> p (a n)"),
                                pt[:].rearrange("p a n -> p (a n)"),
                                masks[:, gg - 2 * t, :],
                                mybir.AluOpType.mult,
                            )
                        first = gg == 0
                        last = gg == ngrp - 1
                        for kk in range(2):
                            jl = 2 * gg + kk
                            jt = b * JTPB + jl
                            for hh in range(2):
                                nc.tensor.matmul(
                                    ot_h[hh][:],
                                    vp[:, jt * 2 + hh, :],
                                    pt[:, 2 * hh + kk, :],
                                    start=(first and kk == 0),
                                    stop=(last and kk == 1),
                                )
                    # normalize: otn[hh*64:(hh+1)*64] = OT * (1/l) broadcast
                    for hh in range(2):
                        linv = lrow.tile([1, 512], F32, tag="linv")
                        nc.vector.reciprocal(linv[:], ot_h[hh][64:65, :])
                        linvr = lrow.tile([1, 512], F32R, tag="linvr")
                        nc.scalar.copy(linvr[:], linv[:])
                        lb = pp.tile([128, 512], F32, tag="pp")
                        nc.tensor.matmul(
                            lb[0:DH, :], ones1[:], linvr[:], start=True, stop=True
                        )
                        otsb = lrow.tile([DH, 512], F32, tag="otsb")
                        nc.scalar.copy(otsb[:], ot_h[hh][0:DH, :])
                        nc.vector.tensor_tensor(
                            otn[hh * DH : (hh + 1) * DH, :],
                            otsb[:],
                            lb[0:DH, :],
                            mybir.AluOpType.mult,
                        )
                    # output projection for this i-tile
                    for c in range(4):
                        for ncol in range(2):
                            op = pp.tile([128, 512], F32, tag="pp")
                            nc.tensor.matmul(
                                op[:],
                                otn[:, c * 128 : (c + 1) * 128],
                                wo[:, ncol * 512 : (ncol + 1) * 512],
                                start=True,
                                stop=True,
                            )
                            osb = outsb.tile([128, 512], F32, tag="osb")
                            nc.vector.tensor_copy(osb[:], op[:])
                            nc.sync.dma_start(
                                out_d[
                                    i0 + c * 128 : i0 + (c + 1) * 128,
                                    ncol * 512 : (ncol + 1) * 512,
                                ],
                                osb[:],
                            )

    _split_waits(nc)
    return nc


_NC = None


def _get_nc():
    global _NC
    if _NC is None:
        _NC = _build()
    return _NC


_RUNNER = None


def _get_runner():
    """Build the sharded PJRT executable once and cache it (bass2jax's
    run_bass_via_pjrt re-jits and reloads the NEFF on every call)."""
    global _RUNNER
    if _RUNNER is not None:
        return _RUNNER
    import jax
    from jax.experimental.shard_map import shard_map
    from jax.sharding import Mesh, PartitionSpec
    from concourse import bass2jax
    from concourse import mybir as _mybir

    nc = _get_nc()
    bass2jax.install_neuronx_cc_hook()
    in_names, out_names, out_avals, zero_shapes = [], [], [], []
    partition_name = (
        nc.partition_id_tensor.name if nc.partition_id_tensor else None
    )
    for alloc in nc.m.functions[0].allocations:
        if not isinstance(alloc, _mybir.MemoryLocationSet):
            continue
        name = alloc.memorylocations[0].name
        if alloc.kind == "ExternalInput":
            if name != partition_name:
                in_names.append(name)
        elif alloc.kind == "ExternalOutput":
            out_names.append(name)
            shape = tuple(alloc.tensor_shape)
            dtype = _mybir.dt.np(alloc.dtype)
            out_avals.append(jax.core.ShapedArray(shape, dtype))
            zero_shapes.append((shape, dtype))
    n_params = len(in_names)
    all_names = in_names + out_names
    if partition_name is not None:
        all_names = all_names + [partition_name]

    def _body(*args):
        operands = list(args)
        if partition_name is not None:
            operands.append(bass2jax.partition_id_tensor())
        outs = bass2jax._bass_exec_p.bind(
            *operands,
            out_avals=tuple(out_avals),
            in_names=tuple(all_names),
            out_names=tuple(out_names),
            lowering_input_output_aliases=(),
            sim_require_finite=True,
            sim_require_nnan=True,
            nc=nc,
        )
        return tuple(outs)

    devices = jax.devices()[:NCORES]
    mesh = Mesh(np.asarray(devices), ("core",))
    in_specs = (PartitionSpec("core"),) * (n_params + len(out_names))
    out_specs = (PartitionSpec("core"),) * len(out_names)
    sharded = jax.jit(
        shard_map(
            _body, mesh=mesh, in_specs=in_specs, out_specs=out_specs,
            check_rep=False,
        ),
        donate_argnums=tuple(range(n_params, n_params + len(out_names))),
        keep_unused=True,
    )

    import jax.numpy as jnp

    sumjit = jax.jit(lambda a: jnp.sum(a.reshape(NCORES, T, D), axis=0))
    _RUNNER = (sharded, sumjit, in_names, out_names, zero_shapes)
    return _RUNNER


def _ramp_masks():
    """masks[v][p, k*512 + f] for group-slot layout [h0j0, h0j1, h1j0, h1j1]:
    slot kk (j-tile offset delta = 512*v + 128*kk): allow f >= delta + p."""
    f = np.arange(512)[None, :]
    p = np.arange(128)[:, None]
    m = np.zeros((2, 128, 4, 512), np.float32)
    for v in range(2):
        for kk in range(2):
            delta = 256 * v + 128 * kk
            allow = (f >= delta + p).astype(np.float32)
            for hh in range(2):
                m[v, :, 2 * hh + kk, :] = allow
    return m.reshape(2, 128, 2048)


def _reference_numpy(x, Wq, bq, Wk, bk, Wv, bv, Wo, bo):
    B_, S_, D_ = x.shape
    d = D_ // H
    x64 = x.astype(np.float64)
    q = (x64 @ Wq + bq).reshape(B_, S_, H, d).transpose(0, 2, 1, 3)
    k = (x64 @ Wk + bk).reshape(B_, S_, H, d).transpose(0, 2, 1, 3)
    v = (x64 @ Wv + bv).reshape(B_, S_, H, d).transpose(0, 2, 1, 3)
    dots = np.einsum("bhid,bhjd->bhij", q, k) * (D_ ** -0.5)
    mask = np.triu(np.ones((S_, S_), bool), k=1)
    dots = np.where(mask, -np.inf, dots)
    dots -= dots.max(axis=-1, keepdims=True)
    e = np.exp(dots)
    attn = e / e.sum(axis=-1, keepdims=True)
    out = np.einsum("bhij,bhjd->bhid", attn, v)
    out = out.transpose(0, 2, 1, 3).reshape(B_, S_, D_)
    return (out @ Wo + bo).astype(np.float32)


def kernel(x, Wq, bq, Wk, bk, Wv, bv, Wo, bo):
    x = np.asarray(x, np.float32)
    Wq, Wk, Wv, Wo = (np.asarray(w, np.float32) for w in (Wq, Wk, Wv, Wo))
    bq, bk, bv, bo = (np.asarray(b_, np.float32) for b_ in (bq, bk, bv, bo))
    if np.any(bq) or np.any(bk) or np.any(bv):
        # projection biases feed the softmax nonlinearly; the fused kernel
        # hardcodes zero biases (as in the problem inputs), so fall back
        return _reference_numpy(x, Wq, bq, Wk, bk, Wv, bv, Wo, bo)

    sharded, sumjit, in_names, out_names, zero_shapes = _get_runner()
    xt = np.ascontiguousarray(x.reshape(T, D).T)
    masks = _ramp_masks()
    ident = np.concatenate([np.eye(DH, dtype=np.float32)] * 2, axis=0)
    onescol = np.ones((128, 1), np.float32)
    ones1 = np.ones((1, DH), np.float32)

    in_maps = []
    for c in range(NCORES):
        cs = slice(c * DC, (c + 1) * DC)
        in_maps.append(
            {
                "xt": xt,
                "wq": np.ascontiguousarray(Wq[:, cs]),
                "wk": np.ascontiguousarray(Wk[:, cs]),
                "wv": np.ascontiguousarray(Wv[:, cs]),
                "wo": np.ascontiguousarray(Wo[cs, :]),
                "mask": masks,
                "ident": ident,
                "onescol": onescol,
                "ones1": ones1,
            }
        )
    concat_in = [
        np.concatenate([m[name] for m in in_maps], axis=0) for name in in_names
    ]
    concat_zeros = [
        np.zeros((NCORES * s[0], *s[1:]), d) for (s, d) in zero_shapes
    ]
    out_arrs = sharded(*concat_in, *concat_zeros)
    try:
        out = np.asarray(sumjit(out_arrs[0]))
    except Exception:
        out = np.asarray(out_arrs[0]).reshape(NCORES, T, D).sum(axis=0)
    out = out + bo
    return out.astype(np.float32).reshape(B, S, D)


# revision 17
# speedup vs baseline: 6.7712x; 4.1108x over previous
"""Causal multi-head attention (B=2, S=2048, D=1024, H=16) on 8 Trainium2
NeuronCores, tensor-parallel over heads (2 heads per core).

Strategy per core c (heads 2c, 2c+1):
  - host pre-transposes x -> xT [D, B*S] and slices Wq/Wk/Wv columns,
    Wo rows for the core.
  - QT = Wq_c^T x^T, KT = Wk_c^T x^T, VT = Wv_c^T x^T   ([128, 4096], fp32r)
  - V' tiles [128 tok, 65] = PE-transpose of VT plus a ones column (the ones
    column makes the PV matmul accumulate the softmax denominator as row 64).
  - ST = K QT per (128-key-tile x 512-query-tile), both heads row-packed on
    the PE array; exp(scale*ST) on ScalarE straight out of PSUM; causal ramp
    masks multiplied in on the diagonal tiles; OT[65,512] += V'^T PT in PSUM.
  - normalize columns by 1/l (l = OT row 64) via reciprocal + K=1 broadcast
    matmul; output projection out_partial = OT^T Wo_c -> [4096, 1024].
  - host sums the 8 partial outputs (row-sharded Wo => partials add up).

All matmuls run in float32r (full PE rate for free dim >= 256, ~2^-13
relative rounding).
"""

import sys

sys.path.insert(0, "/opt/trn_rl_repo")

import numpy as np

import concourse.bass as bass
import concourse.tile as tile
from concourse import mybir
from concourse import bass_utils

F32 = mybir.dt.float32
F32R = mybir.dt.float32r
EXP = mybir.ActivationFunctionType.Exp

B, S, D, H = 2, 2048, 1024, 16
T = B * S                      # 4096 tokens
DH = 64                        # head dim
NCORES = 8
HPC = H // NCORES              # 2 heads per core
DC = HPC * DH                  # 128 dims per core
SCALE = float(D) ** -0.5       # 1/32 (matches the reference's full-dim scale)

NT = T // 512                  # 8 token tiles of 512
JT = T // 128                  # 32 key tiles of 128
ITPB = S // 512                # 4 query tiles per batch
JTPB = S // 128                # 16 key tiles per batch


def _split_waits(nc):
    """This walrus build rejects >1 sync-wait per instruction; hoist extras
    onto same-engine NoOps placed immediately before (engines execute their
    instructions in block order, so semantics are unchanged)."""
    ctr = 0
    for f in nc.m.functions:
        for b in f.blocks:
            out = []
            changed = False
            for inst in b.instructions:
                si = inst.sync_info
                if si is not None:
                    waits = list(si.on_wait)
                    if len(waits) > 1:
                        for w in waits[:-1]:
                            ctr += 1
                            out.append(
                                mybir.InstNoOp(
                                    name=f"waitsplit-{ctr}",
                                    opcode="NoOp",
                                    engine=inst.engine,
                                    ins=[],
                                    outs=[],
                                    sync_info=mybir.SyncInfo(
                                        on_wait=[w], on_update=[]
                                    ),
                                )
                            )
                        inst.sync_info = mybir.SyncInfo(
                            on_wait=waits[-1:], on_update=list(si.on_update)
                        )
                        changed = True
                out.append(inst)
            if changed:
                b.instructions = out


def _build():
    nc = bass.Bass("TRN2", target_bir_lowering=False, debug=False)

    xt_d = nc.dram_tensor("xt", [D, T], F32R, kind="ExternalInput").ap()
    wq_d = nc.dram_tensor("wq", [D, DC], F32R, kind="ExternalInput").ap()
    wk_d = nc.dram_tensor("wk", [D, DC], F32R, kind="ExternalInput").ap()
    wv_d = nc.dram_tensor("wv", [D, DC], F32R, kind="ExternalInput").ap()
    wo_d = nc.dram_tensor("wo", [DC, D], F32R, kind="ExternalInput").ap()
    mask_d = nc.dram_tensor("mask", [2, 128, 2048], F32R, kind="ExternalInput").ap()
    ident_d = nc.dram_tensor("ident", [128, DH], F32, kind="ExternalInput").ap()
    onescol_d = nc.dram_tensor("onescol", [128, 1], F32R, kind="ExternalInput").ap()
    ones1_d = nc.dram_tensor("ones1", [1, DH], F32R, kind="ExternalInput").ap()
    out_d = nc.dram_tensor("out", [T, D], F32, kind="ExternalOutput").ap()

    with tile.TileContext(nc) as tc:
        with (
            tc.tile_pool(name="const", bufs=1) as cpool,
            tc.tile_pool(name="big", bufs=1) as big,
            tc.tile_pool(name="xtp", bufs=2) as xtp,
            tc.tile_pool(name="vstage", bufs=2) as vstage,
            tc.tile_pool(name="ptp", bufs=2) as ptp,
            tc.tile_pool(name="otnp", bufs=2) as otnp,
            tc.tile_pool(name="lrow", bufs=4) as lrow,
            tc.tile_pool(name="outsb", bufs=3) as outsb,
            tc.tile_pool(name="pp", bufs=2, space="PSUM") as pp,
            tc.tile_pool(name="stp", bufs=1, space="PSUM") as stp,
            tc.tile_pool(name="otp", bufs=2, space="PSUM") as otp,
        ):
            # --- constants / weights resident in SBUF
            wq = cpool.tile([128, 8, DC], F32R, tag="wq")
            wk = cpool.tile([128, 8, DC], F32R, tag="wk")
            wv = cpool.tile([128, 8, DC], F32R, tag="wv")
            wo = cpool.tile([DC, D], F32R, tag="wo")
            masks = cpool.tile([128, 2, 2048], F32R, tag="masks")
            ident = cpool.tile([128, DH], F32, tag="ident")
            onescol = cpool.tile([128, 1], F32R, tag="onescol")
            ones1 = cpool.tile([1, DH], F32R, tag="ones1")
            nc.sync.dma_start(wq[:], wq_d.rearrange("(a p) n -> p a n", p=128))
            nc.sync.dma_start(wk[:], wk_d.rearrange("(a p) n -> p a n", p=128))
            nc.sync.dma_start(wv[:], wv_d.rearrange("(a p) n -> p a n", p=128))
            nc.sync.dma_start(wo[:], wo_d[:])
            nc.sync.dma_start(masks[:], mask_d.rearrange("v p n -> p v n"))
            nc.sync.dma_start(ident[:], ident_d[:])
            nc.sync.dma_start(onescol[:], onescol_d[:])
            nc.sync.dma_start(ones1[:], ones1_d[:])

            qt = big.tile([128, T], F32R, tag="qt")
            kt = big.tile([128, T], F32R, tag="kt")
            vp = big.tile([128, 2 * JT, 65], F32R, tag="vp")

            # --- projections: per 512-token tile, QT/KT/VT then V' transposes
            for n in range(NT):
                xt = xtp.tile([128, 8, 512], F32R, tag="xt")
                nc.sync.dma_start(
                    xt[:],
                    xt_d[:, n * 512 : (n + 1) * 512].rearrange(
                        "(a p) n -> p a n", p=128
                    ),
                )
                tok = slice(n * 512, (n + 1) * 512)
                for w_sb, dst in ((wq, qt), (wk, kt)):
                    ps = pp.tile([128, 512], F32, tag="pp")
                    for k in range(8):
                        nc.tensor.matmul(
                            ps[:],
                            w_sb[:, k, :],
                            xt[:, k, :],
                            start=(k == 0),
                            stop=(k == 7),
                        )
                    nc.scalar.copy(dst[:, tok], ps[:])
                ps = pp.tile([128, 512], F32, tag="pp")
                for k in range(8):
                    nc.tensor.matmul(
                        ps[:], wv[:, k, :], xt[:, k, :],
                        start=(k == 0), stop=(k == 7),
                    )
                vt = vstage.tile([128, 512], F32, tag="vt")
                nc.scalar.copy(vt[:], ps[:])
                # V' tiles: transpose each [64,128] block -> [128,64] (+ones)
                for jj in range(4):
                    jt = n * 4 + jj
                    for hh in range(2):
                        tp = pp.tile([128, 512], F32, tag="pp")
                        nc.tensor.transpose(
                            tp[:, 0:DH],
                            vt[hh * DH : (hh + 1) * DH, jj * 128 : (jj + 1) * 128],
                            ident[hh * DH : (hh + 1) * DH, :],
                        )
                        idx = jt * 2 + hh
                        nc.vector.tensor_copy(vp[:, idx, 0:DH], tp[:, 0:DH])
                        nc.vector.tensor_copy(vp[:, idx, DH:65], onescol[:])

            # --- attention + output projection, per batch / query tile
            for b in range(B):
                for t in range(ITPB):
                    g = b * ITPB + t          # global i-tile
                    i0 = g * 512
                    otn = otnp.tile([128, 512], F32R, tag="otn")
                    ot_h = [
                        otp.tile([65, 512], F32, tag="oth", name=f"ot_{g}_{hh}")
                        for hh in range(2)
                    ]
                    ngrp = 2 * (t + 1)        # groups of 2 key tiles
                    for gg in range(ngrp):
                        st = stp.tile([128, 4, 512], F32, tag="st")
                        # ST: both heads row-packed, interleaved issue order
                        for kk in range(2):
                            jl = 2 * gg + kk            # j-tile within batch
                            jt = b * JTPB + jl
                            for hh in range(2):
                                hs = slice(hh * DH, (hh + 1) * DH)
                                nc.tensor.matmul(
                                    st[:, 2 * hh + kk, :],
                                    kt[hs, jt * 128 : (jt + 1) * 128],
                                    qt[hs, i0 : i0 + 512],
                                    start=True,
                                    stop=True,
                                    tile_position=(hh * DH, 0),
                                )
                        pt = ptp.tile([128, 4, 512], F32R, tag="pt")
                        nc.scalar.activation(
                            pt[:].rearrange("p a n -> p (a n)"),
                            st[:].rearrange("p a n -> p (a n)"),
                            EXP,
                            scale=SCALE,
                        )
                        if gg >= 2 * t:  # diagonal group -> causal ramp mask
                            nc.vector.tensor_tensor(
                                pt[:].rearrange("p a n -> p (a n)"),
                                pt[:].rearrange("p a n -> p (a n)"),
                                masks[:, gg - 2 * t, :],
                                mybir.AluOpType.mult,
                            )
                        first = gg == 0
                        last = gg == ngrp - 1
                        for kk in range(2):
                            jl = 2 * gg + kk
                            jt = b * JTPB + jl
                            for hh in range(2):
                                nc.tensor.matmul(
                                    ot_h[hh][:],
                                    vp[:, jt * 2 + hh, :],
                                    pt[:, 2 * hh + kk, :],
                                    start=(first and kk == 0),
                                    stop=(last and kk == 1),
                                )
                    # normalize: otn[hh*64:(hh+1)*64] = OT * (1/l) broadcast
                    for hh in range(2):
                        linv = lrow.tile([1, 512], F32, tag="linv")
                        nc.vector.reciprocal(linv[:], ot_h[hh][64:65, :])
                        linvr = lrow.tile([1, 512], F32R, tag="linvr")
                        nc.scalar.copy(linvr[:], linv[:])
                        lb = pp.tile([128, 512], F32, tag="pp")
                        nc.tensor.matmul(
                            lb[0:DH, :], ones1[:], linvr[:], start=True, stop=True
                        )
                        otsb = lrow.tile([DH, 512], F32, tag="otsb")
                        nc.scalar.copy(otsb[:], ot_h[hh][0:DH, :])
                        nc.vector.tensor_tensor(
                            otn[hh * DH : (hh + 1) * DH, :],
                            otsb[:],
                            lb[0:DH, :],
                            mybir.AluOpType.mult,
                        )
                    # output projection for this i-tile
                    for c in range(4):
                        for ncol in range(2):
                            op = pp.tile([128, 512], F32, tag="pp")
                            nc.tensor.matmul(
                                op[:],
                                otn[:, c * 128 : (c + 1) * 128],
                                wo[:, ncol * 512 : (ncol + 1) * 512],
                                start=True,
                                stop=True,
                            )
                            osb = outsb.tile([128, 512], F32, tag="osb")
                            nc.vector.tensor_copy(osb[:], op[:])
                            nc.sync.dma_start(
                                out_d[
                                    i0 + c * 128 : i0 + (c + 1) * 128,
                                    ncol * 512 : (ncol + 1) * 512,
                                ],
                                osb[:],
                            )

    _split_waits(nc)
    return nc


_NC = None


def _get_nc():
    global _NC
    if _NC is None:
        _NC = _build()
    return _NC


_RUNNER = None
_DEVCACHE = {}


def _get_runner():
    """Build the sharded PJRT executable once and cache it (bass2jax's
    run_bass_via_pjrt re-jits and reloads the NEFF on every call)."""
    global _RUNNER
    if _RUNNER is not None:
        return _RUNNER
    import jax
    from jax.experimental.shard_map import shard_map
    from jax.sharding import Mesh, PartitionSpec
    from concourse import bass2jax
    from concourse import mybir as _mybir

    nc = _get_nc()
    bass2jax.install_neuronx_cc_hook()
    in_names, out_names, out_avals, zero_shapes = [], [], [], []
    partition_name = (
        nc.partition_id_tensor.name if nc.partition_id_tensor else None
    )
    for alloc in nc.m.functions[0].allocations:
        if not isinstance(alloc, _mybir.MemoryLocationSet):
            continue
        name = alloc.memorylocations[0].name
        if alloc.kind == "ExternalInput":
            if name != partition_name:
                in_names.append(name)
        elif alloc.kind == "ExternalOutput":
            out_names.append(name)
            shape = tuple(alloc.tensor_shape)
            dtype = _mybir.dt.np(alloc.dtype)
            out_avals.append(jax.core.ShapedArray(shape, dtype))
            zero_shapes.append((shape, dtype))
    n_params = len(in_names)
    all_names = in_names + out_names
    if partition_name is not None:
        all_names = all_names + [partition_name]

    def _body(*args):
        operands = list(args)
        if partition_name is not None:
            operands.append(bass2jax.partition_id_tensor())
        outs = bass2jax._bass_exec_p.bind(
            *operands,
            out_avals=tuple(out_avals),
            in_names=tuple(all_names),
            out_names=tuple(out_names),
            lowering_input_output_aliases=(),
            sim_require_finite=True,
            sim_require_nnan=True,
            nc=nc,
        )
        return tuple(outs)

    devices = jax.devices()[:NCORES]
    mesh = Mesh(np.asarray(devices), ("core",))
    P = PartitionSpec
    # xt / mask / ident / onescol / ones1 are identical across cores
    # (replicated); Wq/Wk/Wv are column-sharded and Wo row-sharded so the
    # full matrices are passed and XLA distributes the slices.
    spec_by_name = {
        "xt": P(),
        "wq": P(None, "core"),
        "wk": P(None, "core"),
        "wv": P(None, "core"),
        "wo": P("core", None),
        "mask": P(),
        "ident": P(),
        "onescol": P(),
        "ones1": P(),
    }
    in_specs = tuple(spec_by_name[n] for n in in_names) + (P("core"),) * len(
        out_names
    )
    out_specs = (P("core"),) * len(out_names)
    sharded = jax.jit(
        shard_map(
            _body, mesh=mesh, in_specs=in_specs, out_specs=out_specs,
            check_rep=False,
        ),
        donate_argnums=tuple(range(n_params, n_params + len(out_names))),
        keep_unused=True,
    )

    import jax.numpy as jnp
    from jax.sharding import NamedSharding

    sumjit = jax.jit(lambda a: jnp.sum(a.reshape(NCORES, T, D), axis=0))
    # upload once (sharded, one host->device copy total), then all-gather
    # across the on-chip links to replicate without 8x host transfers
    gatherjit = jax.jit(
        lambda a: a,
        out_shardings=NamedSharding(mesh, P()),
    )
    zerojit = jax.jit(
        lambda: tuple(
            jnp.zeros((NCORES * s[0], *s[1:]), d) for (s, d) in zero_shapes
        ),
        out_shardings=tuple(
            NamedSharding(mesh, P("core")) for _ in zero_shapes
        ),
    )
    _RUNNER = (sharded, sumjit, gatherjit, zerojit, mesh, in_names)
    return _RUNNER


def _ramp_masks():
    """masks[v][p, k*512 + f] for group-slot layout [h0j0, h0j1, h1j0, h1j1]:
    slot kk (j-tile offset delta = 512*v + 128*kk): allow f >= delta + p."""
    f = np.arange(512)[None, :]
    p = np.arange(128)[:, None]
    m = np.zeros((2, 128, 4, 512), np.float32)
    for v in range(2):
        for kk in range(2):
            delta = 256 * v + 128 * kk
            allow = (f >= delta + p).astype(np.float32)
            for hh in range(2):
                m[v, :, 2 * hh + kk, :] = allow
    return m.reshape(2, 128, 2048)


def _reference_numpy(x, Wq, bq, Wk, bk, Wv, bv, Wo, bo):
    B_, S_, D_ = x.shape
    d = D_ // H
    x64 = x.astype(np.float64)
    q = (x64 @ Wq + bq).reshape(B_, S_, H, d).transpose(0, 2, 1, 3)
    k = (x64 @ Wk + bk).reshape(B_, S_, H, d).transpose(0, 2, 1, 3)
    v = (x64 @ Wv + bv).reshape(B_, S_, H, d).transpose(0, 2, 1, 3)
    dots = np.einsum("bhid,bhjd->bhij", q, k) * (D_ ** -0.5)
    mask = np.triu(np.ones((S_, S_), bool), k=1)
    dots = np.where(mask, -np.inf, dots)
    dots -= dots.max(axis=-1, keepdims=True)
    e = np.exp(dots)
    attn = e / e.sum(axis=-1, keepdims=True)
    out = np.einsum("bhij,bhjd->bhid", attn, v)
    out = out.transpose(0, 2, 1, 3).reshape(B_, S_, D_)
    return (out @ Wo + bo).astype(np.float32)


def kernel(x, Wq, bq, Wk, bk, Wv, bv, Wo, bo):
    x = np.asarray(x, np.float32)
    Wq, Wk, Wv, Wo = (np.asarray(w, np.float32) for w in (Wq, Wk, Wv, Wo))
    bq, bk, bv, bo = (np.asarray(b_, np.float32) for b_ in (bq, bk, bv, bo))
    if np.any(bq) or np.any(bk) or np.any(bv):
        # projection biases feed the softmax nonlinearly; the fused kernel
        # hardcodes zero biases (as in the problem inputs), so fall back
        return _reference_numpy(x, Wq, bq, Wk, bk, Wv, bv, Wo, bo)

    import jax
    from jax.sharding import NamedSharding, PartitionSpec

    sharded, sumjit, gatherjit, zerojit, mesh, in_names = _get_runner()
    shard0 = NamedSharding(mesh, PartitionSpec("core"))
    rep = NamedSharding(mesh, PartitionSpec())

    if "consts" not in _DEVCACHE:
        masks = _ramp_masks()
        _DEVCACHE["consts"] = {
            "mask": gatherjit(
                jax.device_put(
                    masks.reshape(NCORES, -1),
                    NamedSharding(mesh, PartitionSpec("core", None)),
                )
            ).reshape(2, 128, 2048),
            "ident": jax.device_put(
                np.concatenate([np.eye(DH, dtype=np.float32)] * 2, axis=0), rep
            ),
            "onescol": jax.device_put(np.ones((128, 1), np.float32), rep),
            "ones1": jax.device_put(np.ones((1, DH), np.float32), rep),
        }
    consts = _DEVCACHE["consts"]

    xt = np.ascontiguousarray(x.reshape(T, D).T)
    # one 16MB host->device copy, then an on-device all-gather to replicate
    xt_rep = gatherjit(jax.device_put(xt, shard0))
    arg_by_name = {
        "xt": xt_rep,
        "wq": Wq,
        "wk": Wk,
        "wv": Wv,
        "wo": Wo,
        **consts,
    }
    args = [arg_by_name[name] for name in in_names]
    zeros = zerojit()
    out_arrs = sharded(*args, *zeros)
    out = np.asarray(sumjit(out_arrs[0]))
    out = out + bo
    return out.astype(np.float32).reshape(B, S, D)


# revision 22
# speedup vs baseline: 6.8914x; 1.0178x over previous
"""Causal multi-head attention (B=2, S=2048, D=1024, H=16) on 8 Trainium2
NeuronCores, tensor-parallel over heads (2 heads per core).

Strategy per core c (heads 2c, 2c+1):
  - host pre-transposes x -> xT [D, B*S] and slices Wq/Wk/Wv columns,
    Wo rows for the core.
  - QT = Wq_c^T x^T, KT = Wk_c^T x^T, VT = Wv_c^T x^T   ([128, 4096], fp32r)
  - V' tiles [128 tok, 65] = PE-transpose of VT plus a ones column (the ones
    column makes the PV matmul accumulate the softmax denominator as row 64).
  - ST = K QT per (128-key-tile x 512-query-tile), both heads row-packed on
    the PE array; exp(scale*ST) on ScalarE straight out of PSUM; causal ramp
    masks multiplied in on the diagonal tiles; OT[65,512] += V'^T PT in PSUM.
  - normalize columns by 1/l (l = OT row 64) via reciprocal + K=1 broadcast
    matmul; output projection out_partial = OT^T Wo_c -> [4096, 1024].
  - host sums the 8 partial outputs (row-sharded Wo => partials add up).

All matmuls run in float32r (full PE rate for free dim >= 256, ~2^-13
relative rounding).
"""

import sys

sys.path.insert(0, "/opt/trn_rl_repo")

import numpy as np

import concourse.bass as bass
import concourse.tile as tile
from concourse import mybir
from concourse import bass_utils

F32 = mybir.dt.float32
F32R = mybir.dt.float32r
EXP = mybir.ActivationFunctionType.Exp

B, S, D, H = 2, 2048, 1024, 16
T = B * S                      # 4096 tokens
DH = 64                        # head dim
NCORES = 8
HPC = H // NCORES              # 2 heads per core
DC = HPC * DH                  # 128 dims per core
SCALE = float(D) ** -0.5       # 1/32 (matches the reference's full-dim scale)

NT = T // 512                  # 8 token tiles of 512
JT = T // 128                  # 32 key tiles of 128
ITPB = S // 512                # 4 query tiles per batch
JTPB = S // 128                # 16 key tiles per batch


def _split_waits(nc):
    """This walrus build rejects >1 sync-wait per instruction; hoist extras
    onto same-engine NoOps placed immediately before (engines execute their
    instructions in block order, so semantics are unchanged)."""
    ctr = 0
    for f in nc.m.functions:
        for b in f.blocks:
            out = []
            changed = False
            for inst in b.instructions:
                si = inst.sync_info
                if si is not None:
                    waits = list(si.on_wait)
                    if len(waits) > 1:
                        for w in waits[:-1]:
                            ctr += 1
                            out.append(
                                mybir.InstNoOp(
                                    name=f"waitsplit-{ctr}",
                                    opcode="NoOp",
                                    engine=inst.engine,
                                    ins=[],
                                    outs=[],
                                    sync_info=mybir.SyncInfo(
                                        on_wait=[w], on_update=[]
                                    ),
                                )
                            )
                        inst.sync_info = mybir.SyncInfo(
                            on_wait=waits[-1:], on_update=list(si.on_update)
                        )
                        changed = True
                out.append(inst)
            if changed:
                b.instructions = out


def _build():
    nc = bass.Bass("TRN2", target_bir_lowering=False, debug=False)

    xt_d = nc.dram_tensor("xt", [D, T], F32R, kind="ExternalInput").ap()
    wq_d = nc.dram_tensor("wq", [D, DC], F32R, kind="ExternalInput").ap()
    wk_d = nc.dram_tensor("wk", [D, DC], F32R, kind="ExternalInput").ap()
    wv_d = nc.dram_tensor("wv", [D, DC], F32R, kind="ExternalInput").ap()
    wo_d = nc.dram_tensor("wo", [DC, D], F32R, kind="ExternalInput").ap()
    mask_d = nc.dram_tensor("mask", [4, 128, 1024], F32R, kind="ExternalInput").ap()
    ident_d = nc.dram_tensor("ident", [128, DH], F32, kind="ExternalInput").ap()
    onescol_d = nc.dram_tensor("onescol", [128, 1], F32R, kind="ExternalInput").ap()
    ones1_d = nc.dram_tensor("ones1", [1, DH], F32R, kind="ExternalInput").ap()
    out_d = nc.dram_tensor("out", [T, D], F32, kind="ExternalOutput").ap()

    with tile.TileContext(nc) as tc:
        with (
            tc.tile_pool(name="const", bufs=1) as cpool,
            tc.tile_pool(name="big", bufs=1) as big,
            tc.tile_pool(name="xtp", bufs=2) as xtp,
            tc.tile_pool(name="vstage", bufs=2) as vstage,
            tc.tile_pool(name="ptp", bufs=3) as ptp,
            tc.tile_pool(name="otnp", bufs=2) as otnp,
            tc.tile_pool(name="lrow", bufs=4) as lrow,
            tc.tile_pool(name="outsb", bufs=3) as outsb,
            tc.tile_pool(name="pp", bufs=2, space="PSUM") as pp,
            tc.tile_pool(name="stp", bufs=2, space="PSUM") as stp,
            tc.tile_pool(name="otp", bufs=2, space="PSUM") as otp,
        ):
            # --- constants / weights resident in SBUF
            wq = cpool.tile([128, 8, DC], F32R, tag="wq")
            wk = cpool.tile([128, 8, DC], F32R, tag="wk")
            wv = cpool.tile([128, 8, DC], F32R, tag="wv")
            wo = cpool.tile([DC, D], F32R, tag="wo")
            masks = cpool.tile([128, 4, 1024], F32R, tag="masks")
            ident = cpool.tile([128, DH], F32, tag="ident")
            onescol = cpool.tile([128, 1], F32R, tag="onescol")
            ones1 = cpool.tile([1, DH], F32R, tag="ones1")
            nc.sync.dma_start(wq[:], wq_d.rearrange("(a p) n -> p a n", p=128))
            nc.sync.dma_start(wk[:], wk_d.rearrange("(a p) n -> p a n", p=128))
            nc.sync.dma_start(wv[:], wv_d.rearrange("(a p) n -> p a n", p=128))
            nc.sync.dma_start(ident[:], ident_d[:])
            nc.sync.dma_start(onescol[:], onescol_d[:])
            nc.sync.dma_start(ones1[:], ones1_d[:])

            def emit_late_consts():
                nc.sync.dma_start(wo[:], wo_d[:])
                nc.sync.dma_start(masks[:], mask_d.rearrange("v p n -> p v n"))

            qt = big.tile([128, T], F32R, tag="qt")
            kt = big.tile([128, T], F32R, tag="kt")
            vp = big.tile([128, 2 * JT, 65], F32R, tag="vp")

            def emit_proj(n):
                """QT/KT/VT for token tile n, plus V' transpose tiles."""
                xt = xtp.tile([128, 8, 512], F32R, tag="xt")
                nc.sync.dma_start(
                    xt[:],
                    xt_d[:, n * 512 : (n + 1) * 512].rearrange(
                        "(a p) n -> p a n", p=128
                    ),
                )
                tok = slice(n * 512, (n + 1) * 512)
                for w_sb, dst in ((wq, qt), (wk, kt)):
                    ps = pp.tile([128, 512], F32, tag="pp")
                    for k in range(8):
                        nc.tensor.matmul(
                            ps[:],
                            w_sb[:, k, :],
                            xt[:, k, :],
                            start=(k == 0),
                            stop=(k == 7),
                        )
                    nc.vector.tensor_copy(dst[:, tok], ps[:])
                ps = pp.tile([128, 512], F32, tag="pp")
                for k in range(8):
                    nc.tensor.matmul(
                        ps[:], wv[:, k, :], xt[:, k, :],
                        start=(k == 0), stop=(k == 7),
                    )
                vt = vstage.tile([128, 512], F32, tag="vt")
                nc.vector.tensor_copy(vt[:], ps[:])
                # V' tiles: transpose each [64,128] block -> [128,64] (+ones)
                for jj in range(4):
                    jt = n * 4 + jj
                    for hh in range(2):
                        tp = pp.tile([128, 512], F32, tag="pp")
                        nc.tensor.transpose(
                            tp[:, 0:DH],
                            vt[hh * DH : (hh + 1) * DH, jj * 128 : (jj + 1) * 128],
                            ident[hh * DH : (hh + 1) * DH, :],
                        )
                        idx = jt * 2 + hh
                        nc.vector.tensor_copy(vp[:, idx, 0:DH], tp[:, 0:DH])
                        nc.vector.tensor_copy(vp[:, idx, DH:65], onescol[:])

            def emit_attention(g):
                """attention + normalize + output projection for i-tile g."""
                b, t = divmod(g, ITPB)
                i0 = g * 512
                otn = otnp.tile([128, 512], F32R, tag="otn")
                ot_h = [
                    otp.tile([65, 512], F32, tag="oth", name=f"ot_{g}_{hh}")
                    for hh in range(2)
                ]
                njt = 4 * (t + 1)             # causal: key tiles this i-tile
                for jl in range(njt):
                    jt = b * JTPB + jl
                    st = stp.tile([128, 2, 512], F32, tag="st")
                    for hh in range(2):
                        hs = slice(hh * DH, (hh + 1) * DH)
                        nc.tensor.matmul(
                            st[:, hh, :],
                            kt[hs, jt * 128 : (jt + 1) * 128],
                            qt[hs, i0 : i0 + 512],
                            start=True,
                            stop=True,
                            tile_position=(hh * DH, 0),
                        )
                    pt = ptp.tile([128, 2, 512], F32R, tag="pt")
                    nc.scalar.activation(
                        pt[:].rearrange("p a n -> p (a n)"),
                        st[:].rearrange("p a n -> p (a n)"),
                        EXP,
                        scale=SCALE,
                    )
                    dd = jl - 4 * t
                    if dd >= 0:  # diagonal key tile -> causal ramp mask
                        nc.vector.tensor_tensor(
                            pt[:].rearrange("p a n -> p (a n)"),
                            pt[:].rearrange("p a n -> p (a n)"),
                            masks[:, dd, :],
                            mybir.AluOpType.mult,
                        )
                    for hh in range(2):
                        nc.tensor.matmul(
                            ot_h[hh][:],
                            vp[:, jt * 2 + hh, :],
                            pt[:, hh, :],
                            start=(jl == 0),
                            stop=(jl == njt - 1),
                        )
                # normalize: otn[hh] = OT / broadcast(l); l rides as row 64
                for hh in range(2):
                    linv = lrow.tile([1, 512], F32, tag="linv")
                    nc.vector.reciprocal(linv[:], ot_h[hh][64:65, :])
                    lr = lrow.tile([1, 512], F32R, tag="lr")
                    nc.scalar.copy(lr[:], linv[:])
                    lb = pp.tile([128, 512], F32, tag="pp")
                    nc.tensor.matmul(
                        lb[0:DH, :], ones1[:], lr[:], start=True, stop=True
                    )
                    lbs = lrow.tile([DH, 512], F32, tag="lbs")
                    nc.scalar.copy(lbs[:], lb[0:DH, :])
                    nc.vector.tensor_tensor(
                        otn[hh * DH : (hh + 1) * DH, :],
                        ot_h[hh][0:DH, :],
                        lbs[:],
                        mybir.AluOpType.mult,
                    )
                # output projection for this i-tile
                for c in range(4):
                    for ncol in range(2):
                        op = pp.tile([128, 512], F32, tag="pp")
                        nc.tensor.matmul(
                            op[:],
                            otn[:, c * 128 : (c + 1) * 128],
                            wo[:, ncol * 512 : (ncol + 1) * 512],
                            start=True,
                            stop=True,
                        )
                        osb = outsb.tile([128, 512], F32, tag="osb")
                        nc.vector.tensor_copy(osb[:], op[:])
                        nc.sync.dma_start(
                            out_d[
                                i0 + c * 128 : i0 + (c + 1) * 128,
                                ncol * 512 : (ncol + 1) * 512,
                            ],
                            osb[:],
                        )

            # interleave projection and attention emission so the scheduler
            # can overlap them (attention for i-tile g only needs projection
            # tiles <= g)
            emit_proj(0)
            emit_late_consts()
            emit_attention(0)
            for g in range(1, NT):
                emit_proj(g)
                emit_attention(g)

    _split_waits(nc)
    return nc


_NC = None


def _get_nc():
    global _NC
    if _NC is None:
        _NC = _build()
    return _NC


_RUNNER = None
_DEVCACHE = {}


def _get_runner():
    """Build the sharded PJRT executable once and cache it (bass2jax's
    run_bass_via_pjrt re-jits and reloads the NEFF on every call)."""
    global _RUNNER
    if _RUNNER is not None:
        return _RUNNER
    import jax
    from jax.experimental.shard_map import shard_map
    from jax.sharding import Mesh, PartitionSpec
    from concourse import bass2jax
    from concourse import mybir as _mybir

    nc = _get_nc()
    bass2jax.install_neuronx_cc_hook()
    in_names, out_names, out_avals, zero_shapes = [], [], [], []
    partition_name = (
        nc.partition_id_tensor.name if nc.partition_id_tensor else None
    )
    for alloc in nc.m.functions[0].allocations:
        if not isinstance(alloc, _mybir.MemoryLocationSet):
            continue
        name = alloc.memorylocations[0].name
        if alloc.kind == "ExternalInput":
            if name != partition_name:
                in_names.append(name)
        elif alloc.kind == "ExternalOutput":
            out_names.append(name)
            shape = tuple(alloc.tensor_shape)
            dtype = _mybir.dt.np(alloc.dtype)
            out_avals.append(jax.core.ShapedArray(shape, dtype))
            zero_shapes.append((shape, dtype))
    n_params = len(in_names)
    all_names = in_names + out_names
    if partition_name is not None:
        all_names = all_names + [partition_name]

    def _body(*args):
        operands = list(args)
        if partition_name is not None:
            operands.append(bass2jax.partition_id_tensor())
        outs = bass2jax._bass_exec_p.bind(
            *operands,
            out_avals=tuple(out_avals),
            in_names=tuple(all_names),
            out_names=tuple(out_names),
            lowering_input_output_aliases=(),
            sim_require_finite=True,
            sim_require_nnan=True,
            nc=nc,
        )
        return tuple(outs)

    devices = jax.devices()[:NCORES]
    mesh = Mesh(np.asarray(devices), ("core",))
    P = PartitionSpec
    # xt / mask / ident / onescol / ones1 are identical across cores
    # (replicated); Wq/Wk/Wv are column-sharded and Wo row-sharded so the
    # full matrices are passed and XLA distributes the slices.
    spec_by_name = {
        "xt": P(),
        "wq": P(None, "core"),
        "wk": P(None, "core"),
        "wv": P(None, "core"),
        "wo": P("core", None),
        "mask": P(),
        "ident": P(),
        "onescol": P(),
        "ones1": P(),
    }
    in_specs = tuple(spec_by_name[n] for n in in_names) + (P("core"),) * len(
        out_names
    )
    out_specs = (P("core"),) * len(out_names)
    sharded = jax.jit(
        shard_map(
            _body, mesh=mesh, in_specs=in_specs, out_specs=out_specs,
            check_rep=False,
        ),
        donate_argnums=tuple(range(n_params, n_params + len(out_names))),
        keep_unused=True,
    )

    import jax.numpy as jnp
    from jax.sharding import NamedSharding

    sumjit = jax.jit(lambda a: jnp.sum(a.reshape(NCORES, T, D), axis=0))
    # upload once (sharded, one host->device copy total), then all-gather
    # across the on-chip links to replicate without 8x host transfers
    gatherjit = jax.jit(
        lambda a: a,
        out_shardings=NamedSharding(mesh, P()),
    )
    zerojit = jax.jit(
        lambda: tuple(
            jnp.zeros((NCORES * s[0], *s[1:]), d) for (s, d) in zero_shapes
        ),
        out_shardings=tuple(
            NamedSharding(mesh, P("core")) for _ in zero_shapes
        ),
    )
    _RUNNER = (sharded, sumjit, gatherjit, zerojit, mesh, in_names)
    return _RUNNER


def _ramp_masks():
    """masks[dd][p][hh*512 + f]: causal ramp for the diagonal key tile at
    offset delta = 128*dd from the query tile start: allow f >= delta + p
    (same ramp for both heads)."""
    f = np.arange(512)[None, :]
    p = np.arange(128)[:, None]
    m = np.zeros((4, 128, 2, 512), np.float32)
    for dd in range(4):
        allow = (f >= 128 * dd + p).astype(np.float32)
        for hh in range(2):
            m[dd, :, hh, :] = allow
    return m.reshape(4, 128, 1024)


def _reference_numpy(x, Wq, bq, Wk, bk, Wv, bv, Wo, bo):
    B_, S_, D_ = x.shape
    d = D_ // H
    x64 = x.astype(np.float64)
    q = (x64 @ Wq + bq).reshape(B_, S_, H, d).transpose(0, 2, 1, 3)
    k = (x64 @ Wk + bk).reshape(B_, S_, H, d).transpose(0, 2, 1, 3)
    v = (x64 @ Wv + bv).reshape(B_, S_, H, d).transpose(0, 2, 1, 3)
    dots = np.einsum("bhid,bhjd->bhij", q, k) * (D_ ** -0.5)
    mask = np.triu(np.ones((S_, S_), bool), k=1)
    dots = np.where(mask, -np.inf, dots)
    dots -= dots.max(axis=-1, keepdims=True)
    e = np.exp(dots)
    attn = e / e.sum(axis=-1, keepdims=True)
    out = np.einsum("bhij,bhjd->bhid", attn, v)
    out = out.transpose(0, 2, 1, 3).reshape(B_, S_, D_)
    return (out @ Wo + bo).astype(np.float32)


def kernel(x, Wq, bq, Wk, bk, Wv, bv, Wo, bo):
    x = np.asarray(x, np.float32)
    Wq, Wk, Wv, Wo = (np.asarray(w, np.float32) for w in (Wq, Wk, Wv, Wo))
    bq, bk, bv, bo = (np.asarray(b_, np.float32) for b_ in (bq, bk, bv, bo))
    if np.any(bq) or np.any(bk) or np.any(bv):
        # projection biases feed the softmax nonlinearly; the fused kernel
        # hardcodes zero biases (as in the problem inputs), so fall back
        return _reference_numpy(x, Wq, bq, Wk, bk, Wv, bv, Wo, bo)

    import jax
    from jax.sharding import NamedSharding, PartitionSpec

    sharded, sumjit, gatherjit, zerojit, mesh, in_names = _get_runner()
    shard0 = NamedSharding(mesh, PartitionSpec("core"))
    rep = NamedSharding(mesh, PartitionSpec())

    if "consts" not in _DEVCACHE:
        masks = _ramp_masks()
        _DEVCACHE["consts"] = {
            "mask": gatherjit(
                jax.device_put(
                    masks.reshape(NCORES, -1),
                    NamedSharding(mesh, PartitionSpec("core", None)),
                )
            ).reshape(4, 128, 1024),
            "ident": jax.device_put(
                np.concatenate([np.eye(DH, dtype=np.float32)] * 2, axis=0), rep
            ),
            "onescol": jax.device_put(np.ones((128, 1), np.float32), rep),
            "ones1": jax.device_put(np.ones((1, DH), np.float32), rep),
        }
    consts = _DEVCACHE["consts"]

    xt = np.ascontiguousarray(x.reshape(T, D).T)
    # one 16MB host->device copy, then an on-device all-gather to replicate
    xt_rep = gatherjit(jax.device_put(xt, shard0))
    arg_by_name = {
        "xt": xt_rep,
        "wq": Wq,
        "wk": Wk,
        "wv": Wv,
        "wo": Wo,
        **consts,
    }
    args = [arg_by_name[name] for name in in_names]
    zeros = zerojit()
    out_arrs = sharded(*args, *zeros)
    out = np.asarray(sumjit(out_arrs[0]))
    out = out + bo
    return out.astype(np.float32).reshape(B, S, D)
